# revision 1
# baseline (speedup 1.0000x reference)
"""GCN encoder (2-layer) Bass kernel for Trainium2, 8 NeuronCores.

Strategy (graph/data parallel, per sharding hint):
  - Nodes padded to NPAD=50176 and sharded by contiguous range: core c owns
    destination nodes [c*6272, (c+1)*6272) = 49 blocks of 128.
  - Edges (incl. self-loops) are bucketed by destination block and by source
    half (dma_gather indices are int16, so the feature table is gathered in
    two halves of 25088 rows each). Every (block, half) bucket is padded to a
    uniform tile count TH so all 8 cores run one identical SPMD program.
  - Per layer: h = x @ W (dense matmul, PSUM f32), table hs = h * dinv[src]
    stored in HBM (bf16); per destination block, edge messages are fetched
    with dma_gather (128 edges/tile, edge-major) and segment-summed on the
    TensorEngine via one-hot matmuls: onehot[k,d] = w[k] * (col[k]==d), so
    PSUM[d,f] += sum_k w[k]*hs[src_k][f]. Post: * dinv[dest] + bias (+relu).
  - Layer boundary: hs2 shards are exchanged with an AllGather collective.
  - deg = padded per-node weight lists reduced on DVE; dinv = sqrt(1/deg).

kernel(**inputs) takes the FULL inputs and returns the FULL [50000,128] f32
output; all sharding/gather happens inside.
"""

import sys

sys.path.insert(0, "/opt/trn_rl_repo")

import numpy as np
import ml_dtypes

P = 128
NCORES = 8
BPC = 49                 # dest blocks per core
SHARD = BPC * P          # 6272
NPAD = NCORES * SHARD    # 50176
HALF = NPAD // 2         # 25088
N = 50000
FIN = 256
H = 256                  # layer-1 output width
F2 = 128                 # layer-2 output width
DUMMY_SRC = N + 8        # a zero (pad) node, used as src for pad edges

_BF16 = ml_dtypes.bfloat16


def _preprocess(edge_index, edge_weight):
    """Build all per-core device input arrays from the edge list."""
    row = np.asarray(edge_index[0], dtype=np.int64)
    col = np.asarray(edge_index[1], dtype=np.int64)
    w = np.asarray(edge_weight, dtype=np.float32)

    loop = np.arange(N, dtype=np.int64)
    rows = np.concatenate([row, loop])
    cols = np.concatenate([col, loop])
    ws = np.concatenate([w, np.ones(N, np.float32)])
    EE = rows.shape[0]

    # ---- per-node padded weight lists (for deg on device) ----
    deg_cnt = np.bincount(cols, minlength=NPAD)
    L = int(deg_cnt.max())
    L = (L + 7) & ~7  # round to multiple of 8
    order = np.argsort(cols, kind="stable")
    cs = np.zeros(NPAD + 1, np.int64)
    np.cumsum(deg_cnt, out=cs[1:])
    slot = np.arange(EE) - cs[cols[order]]
    wdeg = np.zeros((NPAD, L), np.float32)
    wdeg[cols[order], slot] = ws[order]
    # partition-major: wdegP[p, nb*L+l] = wdeg[nb*128+p, l]
    wdegP = np.ascontiguousarray(
        wdeg.reshape(NPAD // P, P, L).transpose(1, 0, 2).reshape(P, (NPAD // P) * L)
    )

    # ---- edge streams per (block, half) ----
    blk = cols // P                      # 0..390 (real dests only)
    half = (rows >= HALF).astype(np.int64)
    key = blk * 2 + half
    cnt = np.bincount(key, minlength=(NPAD // P) * 2)
    TH = int(-(-cnt.max() // P))         # tiles per half
    CAP = TH * P
    NB = NPAD // P                       # 392 blocks

    src_a = np.full((NB, 2, CAP), DUMMY_SRC % HALF, np.int16)
    col_a = np.zeros((NB, 2, CAP), np.float32)
    w_a = np.zeros((NB, 2, CAP), np.float32)

    order2 = np.argsort(key, kind="stable")
    cs2 = np.zeros(NB * 2 + 1, np.int64)
    np.cumsum(cnt, out=cs2[1:])
    pos = np.arange(EE) - cs2[key[order2]]
    kb = key[order2] // 2
    kh = key[order2] % 2
    src_sorted = rows[order2]
    src_rel = np.where(kh == 1, src_sorted - HALF, src_sorted).astype(np.int16)
    src_a[kb, kh, pos] = src_rel
    col_a[kb, kh, pos] = (cols[order2] - kb * P).astype(np.float32)
    w_a[kb, kh, pos] = ws[order2]

    # wrapped int16 index layout for dma_gather: index i -> partition i%16,
    # col i//16, replicated across the 8 groups of 16 partitions.
    IW = CAP // 16
    idx_w = src_a.reshape(NB, 2, IW, 16).transpose(0, 1, 3, 2)  # [NB,2,16,IW]
    idx_w = np.ascontiguousarray(np.tile(idx_w, (1, 1, 8, 1)))  # [NB,2,128,IW]

    # col/w in per-tile scalar layout: [.., 128, 2*TH] where slot (h*TH+t)
    # on partition p = edge t*128+p of half h.
    colP = col_a.reshape(NB, 2, TH, P).transpose(3, 0, 1, 2).reshape(P, NB * 2 * TH)
    wfP = w_a.reshape(NB, 2, TH, P).transpose(3, 0, 1, 2).reshape(P, NB * 2 * TH)
    colP = np.ascontiguousarray(colP)
    wfP = np.ascontiguousarray(wfP)

    return dict(L=L, TH=TH, CAP=CAP, wdegP=wdegP, idx_w=idx_w, colP=colP, wfP=wfP)


def _host_golden(x, W1, b1, W2, b2, pp, out_dtype=np.float32, quant=True):
    """Numpy re-implementation of the exact device algorithm (same tiling,
    same bf16 quantization points). For validating the scheme off-device."""
    bf = (lambda a: a.astype(_BF16).astype(np.float32)) if quant else (lambda a: a)
    TH, CAP, L = pp["TH"], pp["CAP"], pp["L"]
    NB = NPAD // P

    wdegP = pp["wdegP"]
    deg = np.zeros(NPAD, np.float32)
    for nb in range(NB):
        blkw = wdegP[:, nb * L:(nb + 1) * L]
        deg[nb * P:(nb + 1) * P] = blkw.sum(axis=1)
    dinv = np.sqrt(1.0 / (deg + (deg == 0)))

    xp = np.zeros((NPAD, FIN), np.float32)
    xp[:N] = x
    h1 = bf(xp) @ bf(W1)                     # bf16 inputs, f32 accum
    hs1 = bf(h1 * dinv[:, None])             # stored bf16

    idx_w = pp["idx_w"]; colP = pp["colP"]; wfP = pp["wfP"]
    out1 = np.zeros((NPAD, H), np.float32)
    for nb in range(NB):
        acc = np.zeros((P, H), np.float32)
        for hh in range(2):
            iw = idx_w[nb, hh, :16, :]                      # [16, IW]
            flat = iw.T.reshape(-1)[:CAP].astype(np.int64)  # unwrap
            base = 0 if hh == 0 else HALF
            msgs = hs1[base + flat]                         # [CAP, H]
            for t in range(TH):
                oh = np.zeros((P, P), np.float32)
                c = colP[:, (nb * 2 + hh) * TH + t]
                wv = bf(wfP[:, (nb * 2 + hh) * TH + t])
                oh[np.arange(P), c.astype(np.int64)] = wv
                acc += oh.T @ msgs[t * P:(t + 1) * P]
        z = acc * dinv[nb * P:(nb + 1) * P, None] + b1[None, :]
        out1[nb * P:(nb + 1) * P] = np.maximum(z, 0.0)

    h2in = bf(out1)
    h2 = h2in @ bf(W2)
    hs2 = bf(h2 * dinv[:, None])

    out2 = np.zeros((NPAD, F2), np.float32)
    for nb in range(NB):
        acc = np.zeros((P, F2), np.float32)
        for hh in range(2):
            iw = idx_w[nb, hh, :16, :]
            flat = iw.T.reshape(-1)[:CAP].astype(np.int64)
            base = 0 if hh == 0 else HALF
            msgs = hs2[base + flat]
            for t in range(TH):
                oh = np.zeros((P, P), np.float32)
                c = colP[:, (nb * 2 + hh) * TH + t]
                wv = bf(wfP[:, (nb * 2 + hh) * TH + t])
                oh[np.arange(P), c.astype(np.int64)] = wv
                acc += oh.T @ msgs[t * P:(t + 1) * P]
        out2[nb * P:(nb + 1) * P] = (
            acc * dinv[nb * P:(nb + 1) * P, None] + b2[None, :]
        )
    return out2[:N].astype(out_dtype)


# ---------------------------------------------------------------------------
# Bass device kernel
# ---------------------------------------------------------------------------

_NC_CACHE = {}


def _build_nc(TH, L):
    import concourse.bass as bass  # noqa: F401
    import concourse.mybir as mybir
    import concourse.tile as tile
    from concourse import bacc
    from concourse.library_config import mlp

    DT = mybir.dt.bfloat16
    F32 = mybir.dt.float32
    I16 = mybir.dt.int16
    AL = mybir.AluOpType
    AF = mybir.ActivationFunctionType
    AX = mybir.AxisListType

    CAP = TH * P
    IW = CAP // 16
    NB = NPAD // P           # 392
    NBC = 56                 # wdeg chunk: blocks per chunk (392 = 7*56)

    nc = bacc.Bacc("TRN2", target_bir_lowering=False, debug=True,
                   num_devices=NCORES)
    xt3_d = nc.dram_tensor("xt3", [2, P, NPAD], DT, kind="ExternalInput")
    w1_d = nc.dram_tensor("w1c", [2, P, H], DT, kind="ExternalInput")
    w2_d = nc.dram_tensor("w2c", [2, P, F2], DT, kind="ExternalInput")
    b1_d = nc.dram_tensor("b1f", [P, H], F32, kind="ExternalInput")
    b2_d = nc.dram_tensor("b2f", [P, F2], F32, kind="ExternalInput")
    iota_d = nc.dram_tensor("iota", [P, P], F32, kind="ExternalInput")
    wdeg_d = nc.dram_tensor("wdegP", [P, NB * L], F32, kind="ExternalInput")
    wdegl_d = nc.dram_tensor("wdeglP", [P, BPC * L], F32, kind="ExternalInput")
    idx_d = nc.dram_tensor("idxP", [P, BPC * 2 * IW], I16, kind="ExternalInput")
    col_d = nc.dram_tensor("colP", [P, BPC * 2 * TH], F32, kind="ExternalInput")
    wf_d = nc.dram_tensor("wfP", [P, BPC * 2 * TH], F32, kind="ExternalInput")
    out_d = nc.dram_tensor("out2", [SHARD, F2], F32, kind="ExternalOutput")

    with tile.TileContext(nc) as tc:
        with (
            tc.tile_pool(name="dram", bufs=1, space="DRAM") as dpool,
            tc.tile_pool(name="const", bufs=1) as cpool,
            tc.tile_pool(name="wdegc", bufs=2) as wpool,
            tc.tile_pool(name="xs", bufs=3) as xpool,
            tc.tile_pool(name="hst", bufs=3) as hpool,
            tc.tile_pool(name="msg", bufs=2) as mpool,
            tc.tile_pool(name="oh", bufs=8) as ohpool,
            tc.tile_pool(name="post", bufs=3) as tpool,
            tc.tile_pool(name="ph1", bufs=2, space="PSUM") as ph1p,
            tc.tile_pool(name="pagg", bufs=2, space="PSUM") as paggp,
            tc.tile_pool(name="pc", bufs=2, space="PSUM") as pcp,
        ):
            hs1_tab = dpool.tile([NPAD, H], DT)
            h2in_dram = dpool.tile([SHARD, H], DT)
            hs2_shard = dpool.tile([SHARD, F2], DT)
            hs2_full = dpool.tile([NPAD, F2], DT, addr_space="Shared")

            nc.gpsimd.load_library(mlp)

            # ---- constants ----
            w1_sb = cpool.tile([P, 2 * H], DT)
            nc.sync.dma_start(out=w1_sb[:, 0:H], in_=w1_d[0])
            nc.sync.dma_start(out=w1_sb[:, H:2 * H], in_=w1_d[1])
            w2_sb = cpool.tile([P, 2 * F2], DT)
            nc.sync.dma_start(out=w2_sb[:, 0:F2], in_=w2_d[0])
            nc.sync.dma_start(out=w2_sb[:, F2:2 * F2], in_=w2_d[1])
            b1_sb = cpool.tile([P, H], F32)
            nc.sync.dma_start(out=b1_sb[:], in_=b1_d[:])
            b2_sb = cpool.tile([P, F2], F32)
            nc.sync.dma_start(out=b2_sb[:], in_=b2_d[:])
            iota_sb = cpool.tile([P, P], F32)
            nc.sync.dma_start(out=iota_sb[:], in_=iota_d[:])
            idx_sb = cpool.tile([P, BPC * 2 * IW], I16)
            nc.sync.dma_start(out=idx_sb[:], in_=idx_d[:])
            col_sb = cpool.tile([P, BPC * 2 * TH], F32)
            nc.sync.dma_start(out=col_sb[:], in_=col_d[:])
            wf_sb = cpool.tile([P, BPC * 2 * TH], F32)
            nc.sync.dma_start(out=wf_sb[:], in_=wf_d[:])

            # ---- deg -> dinv (full, and local shard) ----
            deg_sb = cpool.tile([P, NB], F32)
            for ch in range(NB // NBC):
                wt = wpool.tile([P, NBC * L], F32, tag="wdeg")
                nc.sync.dma_start(out=wt[:], in_=wdeg_d[:, ch * NBC * L:(ch + 1) * NBC * L])
                nc.vector.reduce_sum(
                    deg_sb[:, ch * NBC:(ch + 1) * NBC],
                    wt[:].rearrange("p (nb l) -> p nb l", l=L),
                    axis=AX.X,
                )
            eq_sb = cpool.tile([P, NB], F32)
            nc.vector.tensor_scalar(eq_sb[:], deg_sb[:], 0.0, None, AL.is_equal)
            nc.vector.tensor_tensor(deg_sb[:], deg_sb[:], eq_sb[:], AL.add)
            rec_sb = cpool.tile([P, NB], F32)
            nc.vector.reciprocal(rec_sb[:], deg_sb[:])
            dinv_sb = cpool.tile([P, NB], F32)
            nc.scalar.sqrt(dinv_sb[:], rec_sb[:])

            wl_sb = cpool.tile([P, BPC * L], F32)
            nc.sync.dma_start(out=wl_sb[:], in_=wdegl_d[:])
            degl_sb = cpool.tile([P, BPC], F32)
            nc.vector.reduce_sum(
                degl_sb[:], wl_sb[:].rearrange("p (nb l) -> p nb l", l=L), axis=AX.X
            )
            eql_sb = cpool.tile([P, BPC], F32)
            nc.vector.tensor_scalar(eql_sb[:], degl_sb[:], 0.0, None, AL.is_equal)
            nc.vector.tensor_tensor(degl_sb[:], degl_sb[:], eql_sb[:], AL.add)
            recl_sb = cpool.tile([P, BPC], F32)
            nc.vector.reciprocal(recl_sb[:], degl_sb[:])
            dinvl_sb = cpool.tile([P, BPC], F32)
            nc.scalar.sqrt(dinvl_sb[:], recl_sb[:])

            # ---- phase A: h1 = x @ W1 (all nodes), hs1 = h1 * dinv ----
            for s in range(NPAD // 512):
                xa = xpool.tile([P, 512], DT, tag="xa")
                xb = xpool.tile([P, 512], DT, tag="xb")
                nc.sync.dma_start(out=xa[:], in_=xt3_d[0][:, s * 512:(s + 1) * 512])
                nc.sync.dma_start(out=xb[:], in_=xt3_d[1][:, s * 512:(s + 1) * 512])
                for q in range(4):
                    nb = s * 4 + q
                    ph = ph1p.tile([P, H], F32)
                    nc.tensor.matmul(ph[:], lhsT=xa[:, q * P:(q + 1) * P],
                                     rhs=w1_sb[:, 0:H], start=True, stop=False)
                    nc.tensor.matmul(ph[:], lhsT=xb[:, q * P:(q + 1) * P],
                                     rhs=w1_sb[:, H:2 * H], start=False, stop=True)
                    hst = hpool.tile([P, H], DT, tag="hst")
                    nc.scalar.activation(hst[:], ph[:], AF.Copy,
                                         scale=dinv_sb[:, nb:nb + 1])
                    nc.sync.dma_start(out=hs1_tab[nb * P:(nb + 1) * P, :], in_=hst[:])

            # ---- phase B: layer-1 aggregation per dest block ----
            for b in range(BPC):
                msgs = []
                for hh in range(2):
                    m = mpool.tile([P, TH, H], DT, tag=f"msg{hh}")
                    src = hs1_tab[0:HALF, :] if hh == 0 else hs1_tab[HALF:NPAD, :]
                    nc.gpsimd.dma_gather(
                        m[:], src, idx_sb[:, (b * 2 + hh) * IW:(b * 2 + hh + 1) * IW],
                        CAP, CAP, H, single_packet=False)
                    msgs.append(m)
                pagg = paggp.tile([P, H], F32)
                for t in range(2 * TH):
                    hh, tt = (0, t) if t < TH else (1, t - TH)
                    oh = ohpool.tile([P, P], DT, tag="oh")
                    sc = (b * 2 + hh) * TH + tt
                    nc.vector.tensor_scalar(oh[:], iota_sb[:], col_sb[:, sc:sc + 1],
                                            wf_sb[:, sc:sc + 1], AL.is_equal, AL.mult)
                    nc.tensor.matmul(pagg[:], lhsT=oh[:], rhs=msgs[hh][:, tt, :],
                                     start=(t == 0), stop=(t == 2 * TH - 1))
                t1 = tpool.tile([P, H], F32, tag="t1")
                nc.vector.tensor_scalar(t1[:], pagg[:], dinvl_sb[:, b:b + 1], None,
                                        AL.mult)
                t2 = tpool.tile([P, H], F32, tag="t2")
                nc.vector.tensor_tensor(t2[:], t1[:], b1_sb[:], AL.add)
                rl = hpool.tile([P, H], DT, tag="rl")
                nc.scalar.activation(rl[:], t2[:], AF.Relu)
                nc.sync.dma_start(out=h2in_dram[b * P:(b + 1) * P, :], in_=rl[:])

            # ---- phase C: h2 = relu_out @ W2, hs2 = h2 * dinv (own shard) ----
            for b in range(BPC):
                ph2 = pcp.tile([P, F2], F32, tag="pc")
                for c2 in range(2):
                    at = ohpool.tile([P, P], DT, tag="at")
                    nc.sync.dma_start(
                        out=at[:],
                        in_=h2in_dram[b * P:(b + 1) * P, c2 * P:(c2 + 1) * P],
                        transpose=True)
                    nc.tensor.matmul(ph2[:], lhsT=at[:],
                                     rhs=w2_sb[:, c2 * F2:(c2 + 1) * F2],
                                     start=(c2 == 0), stop=(c2 == 1))
                hsb = hpool.tile([P, F2], DT, tag="hsb")
                nc.scalar.activation(hsb[:], ph2[:], AF.Copy,
                                     scale=dinvl_sb[:, b:b + 1])
                nc.sync.dma_start(out=hs2_shard[b * P:(b + 1) * P, :], in_=hsb[:])

            # ---- phase D: exchange hs2 shards ----
            nc.gpsimd.collective_compute(
                "AllGather", AL.bypass,
                replica_groups=[list(range(NCORES))],
                ins=[hs2_shard[:]],
                outs=[hs2_full[:]],
            )

            # ---- phase E: layer-2 aggregation per dest block ----
            for b in range(BPC):
                msgs = []
                for hh in range(2):
                    m = mpool.tile([P, TH, F2], DT, tag=f"msg{hh}")
                    src = hs2_full[0:HALF, :] if hh == 0 else hs2_full[HALF:NPAD, :]
                    nc.gpsimd.dma_gather(
                        m[:], src, idx_sb[:, (b * 2 + hh) * IW:(b * 2 + hh + 1) * IW],
                        CAP, CAP, F2, single_packet=False)
                    msgs.append(m)
                pagg2 = pcp.tile([P, F2], F32, tag="pc")
                for t in range(2 * TH):
                    hh, tt = (0, t) if t < TH else (1, t - TH)
                    oh = ohpool.tile([P, P], DT, tag="oh")
                    sc = (b * 2 + hh) * TH + tt
                    nc.vector.tensor_scalar(oh[:], iota_sb[:], col_sb[:, sc:sc + 1],
                                            wf_sb[:, sc:sc + 1], AL.is_equal, AL.mult)
                    nc.tensor.matmul(pagg2[:], lhsT=oh[:], rhs=msgs[hh][:, tt, :],
                                     start=(t == 0), stop=(t == 2 * TH - 1))
                o1 = tpool.tile([P, F2], F32, tag="o1")
                nc.vector.tensor_scalar(o1[:], pagg2[:], dinvl_sb[:, b:b + 1], None,
                                        AL.mult)
                o2 = tpool.tile([P, F2], F32, tag="o2")
                nc.vector.tensor_tensor(o2[:], o1[:], b2_sb[:], AL.add)
                nc.sync.dma_start(out=out_d[b * P:(b + 1) * P, :], in_=o2[:])

    nc.compile()
    return nc


def _make_inputs(x, W1, b1, W2, b2, pp):
    """Per-core input maps."""
    TH, L = pp["TH"], pp["L"]
    IW = (TH * P) // 16
    NB = NPAD // P

    xp = np.zeros((NPAD, FIN), np.float32)
    xp[:N] = x
    xt3 = np.ascontiguousarray(
        xp.T.reshape(2, P, NPAD).astype(_BF16))
    w1c = np.ascontiguousarray(W1.reshape(2, P, H).astype(_BF16))
    w2c = np.ascontiguousarray(W2.reshape(2, P, F2).astype(_BF16))
    b1f = np.ascontiguousarray(np.tile(b1[None, :], (P, 1)).astype(np.float32))
    b2f = np.ascontiguousarray(np.tile(b2[None, :], (P, 1)).astype(np.float32))
    iota = np.tile(np.arange(P, dtype=np.float32)[None, :], (P, 1))

    wdegP = pp["wdegP"]
    idx_w = pp["idx_w"]        # [NB, 2, 128, IW]
    colP = pp["colP"]          # [128, NB*2*TH]
    wfP = pp["wfP"]

    in_maps = []
    for c in range(NCORES):
        b0 = c * BPC
        idxP = np.ascontiguousarray(
            idx_w[b0:b0 + BPC].transpose(2, 0, 1, 3).reshape(P, BPC * 2 * IW))
        in_maps.append({
            "xt3": xt3,
            "w1c": w1c,
            "w2c": w2c,
            "b1f": b1f,
            "b2f": b2f,
            "iota": iota,
            "wdegP": wdegP,
            "wdeglP": np.ascontiguousarray(wdegP[:, b0 * L:(b0 + BPC) * L]),
            "idxP": idxP,
            "colP": np.ascontiguousarray(colP[:, b0 * 2 * TH:(b0 + BPC) * 2 * TH]),
            "wfP": np.ascontiguousarray(wfP[:, b0 * 2 * TH:(b0 + BPC) * 2 * TH]),
        })
    return in_maps


def kernel(x, edge_index, edge_weight, W1, b1, W2, b2, _trace=False):
    from concourse.bass_utils import run_bass_kernel_spmd

    x = np.asarray(x, dtype=np.float32)
    W1 = np.asarray(W1, dtype=np.float32)
    b1 = np.asarray(b1, dtype=np.float32)
    W2 = np.asarray(W2, dtype=np.float32)
    b2 = np.asarray(b2, dtype=np.float32)

    pp = _preprocess(np.asarray(edge_index), np.asarray(edge_weight))
    key = (pp["TH"], pp["L"])
    if key not in _NC_CACHE:
        _NC_CACHE[key] = _build_nc(*key)
    nc = _NC_CACHE[key]

    in_maps = _make_inputs(x, W1, b1, W2, b2, pp)
    res = run_bass_kernel_spmd(nc, in_maps, list(range(NCORES)), trace=_trace)
    out = np.concatenate([res.results[c]["out2"] for c in range(NCORES)], axis=0)
    if _trace:
        kernel._last_result = res
    return np.ascontiguousarray(out[:N])



# revision 5
# speedup vs baseline: 2.0269x; 2.0269x over previous
"""GCN encoder (2-layer) Bass kernel for Trainium2, 8 NeuronCores.

Strategy (graph/data parallel; dest nodes sharded, contiguous ranges):
  - Nodes padded to NPAD=50176; core c owns dest blocks [c*49, (c+1)*49),
    49 blocks of 128 dests each.
  - Edges (incl. self-loops) bucketed by (dest block, src half); each bucket
    padded to TH tiles of 128 edge slots. All normalization (dinv[src] *
    w * dinv[dst]) is folded into host-precomputed one-hot tiles
    oh[slot, dst_col] (bf16), shared by both layers.
  - Layer 1 needs no runtime gather: x is a kernel input, so the per-edge
    source rows xg[slot] = x[src] are gathered ON HOST and streamed as
    dense tiles. Per dest block: aggT[f, d] = sum_t xg_tile^T oh_tile
    (PE, 2 fin chunks), then out1T[h, d] = W1c^T aggT (PE), bias+relu on
    ACT (b1 is per-partition in this transposed layout), then
    hs2[d, f2] = reluT^T W2c (PE). No transposes needed anywhere.
  - hs2 shards exchanged with AllGather; layer 2 fetches per-edge rows with
    dma_gather (128 rows/tile) and aggregates with the same one-hot tiles:
    out2T[f2, d] = sum msg^T oh. Output is written f2-major and transposed
    on host.

kernel(**inputs) takes FULL inputs, returns the FULL [50000,128] f32 output.
"""

import sys

sys.path.insert(0, "/opt/trn_rl_repo")

import numpy as np
import ml_dtypes

P = 128
NCORES = 8
BPC = 49                 # dest blocks per core
SHARD = BPC * P          # 6272
NPAD = NCORES * SHARD    # 50176
HALF = NPAD // 2         # 25088
NB = NPAD // P           # 392 dest blocks
N = 50000
FIN = 256
H = 256                  # layer-1 output width
F2 = 128                 # layer-2 output width

_BF16 = ml_dtypes.bfloat16


def _preprocess(edge_index, edge_weight):
    """Edge bucketing + all graph-structure-derived device arrays."""
    row = np.asarray(edge_index[0], dtype=np.int64)
    col = np.asarray(edge_index[1], dtype=np.int64)
    w = np.asarray(edge_weight, dtype=np.float32)

    loop = np.arange(N, dtype=np.int64)
    rows = np.concatenate([row, loop])
    cols = np.concatenate([col, loop])
    ws = np.concatenate([w, np.ones(N, np.float32)])
    EE = rows.shape[0]

    deg = np.bincount(cols, weights=ws.astype(np.float64), minlength=NPAD)
    deg = deg.astype(np.float32)
    dinv = np.where(deg > 0, 1.0 / np.sqrt(np.maximum(deg, 1e-30)), 0.0)
    dinv = dinv.astype(np.float32)
    wfull = (dinv[rows] * ws * dinv[cols]).astype(np.float32)

    blk = cols // P
    half = (rows >= HALF).astype(np.int64)
    key = blk * 2 + half
    cnt = np.bincount(key, minlength=NB * 2)
    TH = int(-(-cnt.max() // P))
    CAP = TH * P

    order = np.argsort(key, kind="stable")
    cs = np.zeros(NB * 2 + 1, np.int64)
    np.cumsum(cnt, out=cs[1:])
    pos = np.arange(EE) - cs[key[order]]

    kb = key[order]                       # bucket id, sorted
    src_s = rows[order]
    col_s = (cols[order] % P).astype(np.int64)
    w_s = wfull[order]

    # per-slot arrays, bucket-major: slot = kb*CAP + pos
    src_slot = np.full(NB * 2 * CAP, -1, np.int64)
    src_slot[kb * CAP + pos] = src_s
    colw_col = np.zeros(NB * 2 * CAP, np.int64)
    colw_col[kb * CAP + pos] = col_s
    w_slot = np.zeros(NB * 2 * CAP, np.float32)
    w_slot[kb * CAP + pos] = w_s

    # one-hot tiles [NB*2*TH, 128 slot, 128 dst] bf16
    oh = np.zeros((NB * 2 * TH, P, P), _BF16)
    tidx = np.arange(NB * 2 * CAP) // P      # global tile id of each slot
    pslot = np.arange(NB * 2 * CAP) % P
    real = src_slot >= 0
    oh[tidx[real], pslot[real], colw_col[real]] = w_slot[real].astype(_BF16)

    # gather indices, int16, relative to half, wrapped [NB,2,128,IW]
    IW = CAP // 16
    src_rel = np.where(src_slot >= 0,
                       np.where(src_slot >= HALF, src_slot - HALF, src_slot),
                       0).astype(np.int16)
    idx_w = src_rel.reshape(NB, 2, IW, 16).transpose(0, 1, 3, 2)
    idx_w = np.ascontiguousarray(np.tile(idx_w, (1, 1, 8, 1)))  # [NB,2,128,IW]

    return dict(TH=TH, CAP=CAP, oh=oh, src_slot=src_slot, idx_w=idx_w,
                dinv=dinv)


def _host_golden(x, W1, b1, W2, b2, pp):
    """Numpy re-implementation of the device algorithm with the same bf16
    quantization points, for off-device validation."""
    bf = lambda a: a.astype(_BF16).astype(np.float32)
    TH, CAP = pp["TH"], pp["CAP"]
    oh = pp["oh"].astype(np.float32)         # [NB*2*TH, 128, 128]
    src = pp["src_slot"].reshape(NB, 2 * CAP)

    xp = np.zeros((NPAD, FIN), np.float32)
    xp[:N] = x
    xbf = bf(xp)
    W1b, W2b = bf(W1), bf(W2)

    hs2 = np.zeros((NPAD, F2), np.float32)
    for nb in range(NB):
        aggT = np.zeros((FIN, P), np.float32)
        for t in range(2 * TH):
            sl = src[nb, t * P:(t + 1) * P]
            xg = xbf[np.clip(sl, 0, None)]          # [128, 256]
            o = oh[nb * 2 * TH + t]                 # [128 slot, 128 dst]
            aggT += xg.T @ o
        aggT = bf(aggT)
        out1T = W1b.T @ aggT + b1[:, None]          # [256 h, 128 d]
        reluT = bf(np.maximum(out1T, 0.0))
        hs2[nb * P:(nb + 1) * P] = bf(reluT.T @ W2b)

    hs2b = bf(hs2)
    out = np.zeros((NPAD, F2), np.float32)
    for nb in range(NB):
        out2T = np.zeros((F2, P), np.float32)
        for t in range(2 * TH):
            sl = src[nb, t * P:(t + 1) * P]
            msg = hs2b[np.clip(sl, 0, None)]        # [128, 128]
            o = oh[nb * 2 * TH + t]
            out2T += msg.T @ o
        out[nb * P:(nb + 1) * P] = out2T.T + b2[None, :]
    return out[:N].astype(np.float32)


# ---------------------------------------------------------------------------
# Bass device kernel
# ---------------------------------------------------------------------------

_NC_CACHE = {}


def _build_nc(TH):
    import concourse.bass as bass  # noqa: F401
    import concourse.mybir as mybir
    import concourse.tile as tile
    from concourse import bacc
    from concourse.library_config import mlp

    DT = mybir.dt.bfloat16
    F32 = mybir.dt.float32
    I16 = mybir.dt.int16
    AF = mybir.ActivationFunctionType
    AL = mybir.AluOpType

    CAP = TH * P
    IW = CAP // 16
    NT = 2 * TH              # tiles per dest block

    nc = bacc.Bacc("TRN2", target_bir_lowering=False, debug=True,
                   num_devices=NCORES)
    xg_d = nc.dram_tensor("xg", [P, BPC * NT * FIN], DT, kind="ExternalInput")
    oh_d = nc.dram_tensor("oh", [P, BPC * NT * P], DT, kind="ExternalInput")
    idx_d = nc.dram_tensor("idxP", [P, BPC * 2 * IW], I16, kind="ExternalInput")
    w1_d = nc.dram_tensor("w1c", [P, 2 * 2 * P], DT, kind="ExternalInput")
    w2_d = nc.dram_tensor("w2c", [P, 2 * F2], DT, kind="ExternalInput")
    b1_d = nc.dram_tensor("b1c", [P, 2], F32, kind="ExternalInput")
    b2_d = nc.dram_tensor("b2c", [P, 1], F32, kind="ExternalInput")
    out_d = nc.dram_tensor("out2T", [P, BPC * P], F32, kind="ExternalOutput")

    with tile.TileContext(nc) as tc:
        with (
            tc.tile_pool(name="dram", bufs=1, space="DRAM") as dpool,
            tc.tile_pool(name="const", bufs=1) as cpool,
            tc.tile_pool(name="xgs", bufs=2) as xpool,
            tc.tile_pool(name="ohs", bufs=2) as opool,
            tc.tile_pool(name="msg", bufs=2) as mpool,
            tc.tile_pool(name="sb", bufs=3) as spool,
            tc.tile_pool(name="pagg", bufs=4, space="PSUM") as pagg,
            tc.tile_pool(name="po1", bufs=2, space="PSUM") as po1,
            tc.tile_pool(name="psm", bufs=2, space="PSUM") as psm,
        ):
            hs2_shard = dpool.tile([SHARD, F2], DT)
            hs2_full = dpool.tile([NPAD, F2], DT, addr_space="Shared")

            nc.gpsimd.load_library(mlp)

            # ---- constants ----
            w1_sb = cpool.tile([P, 2, 2, P], DT)     # [fin_c, h_c]
            nc.sync.dma_start(out=w1_sb[:], in_=w1_d[:])
            w2_sb = cpool.tile([P, 2, F2], DT)       # [h_c]
            nc.sync.dma_start(out=w2_sb[:], in_=w2_d[:])
            b1_sb = cpool.tile([P, 2], F32)
            nc.sync.dma_start(out=b1_sb[:], in_=b1_d[:])
            b2_sb = cpool.tile([P, 1], F32)
            nc.sync.dma_start(out=b2_sb[:], in_=b2_d[:])
            idx_sb = cpool.tile([P, BPC * 2 * IW], I16)
            nc.sync.dma_start(out=idx_sb[:], in_=idx_d[:])

            # ---- layer 1 + layer-2 table, per dest block ----
            for b in range(BPC):
                xg = xpool.tile([P, NT, FIN], DT, tag="xg")
                nc.sync.dma_start(
                    out=xg[:], in_=xg_d[:, b * NT * FIN:(b + 1) * NT * FIN])
                oh = opool.tile([P, NT, P], DT, tag="oh")
                nc.sync.dma_start(
                    out=oh[:], in_=oh_d[:, b * NT * P:(b + 1) * NT * P])

                aggT0 = pagg.tile([P, P], F32, tag="aggT")
                aggT1 = pagg.tile([P, P], F32, tag="aggT")
                aggTp = [aggT0, aggT1]
                for t in range(NT):
                    for c in range(2):
                        nc.tensor.matmul(
                            aggTp[c][:],
                            lhsT=xg[:, t, c * P:(c + 1) * P],
                            rhs=oh[:, t, :],
                            start=(t == 0), stop=(t == NT - 1))
                aggT_sb = spool.tile([P, 2, P], DT, tag="aggT_sb")
                for c in range(2):
                    nc.vector.tensor_copy(aggT_sb[:, c, :], aggTp[c][:])

                reluT_sb = spool.tile([P, 2, P], DT, tag="reluT")
                for hc in range(2):
                    o1 = po1.tile([P, P], F32, tag="o1")
                    for c in range(2):
                        nc.tensor.matmul(
                            o1[:], lhsT=w1_sb[:, c, hc, :],
                            rhs=aggT_sb[:, c, :],
                            start=(c == 0), stop=(c == 1))
                    nc.scalar.activation(reluT_sb[:, hc, :], o1[:], AF.Relu,
                                         bias=b1_sb[:, hc:hc + 1])

                ph = psm.tile([P, F2], F32, tag="sm")
                for hc in range(2):
                    nc.tensor.matmul(ph[:], lhsT=reluT_sb[:, hc, :],
                                     rhs=w2_sb[:, hc, :],
                                     start=(hc == 0), stop=(hc == 1))
                hsb = spool.tile([P, F2], DT, tag="hsb")
                nc.vector.tensor_copy(hsb[:], ph[:])
                nc.sync.dma_start(out=hs2_shard[b * P:(b + 1) * P, :],
                                  in_=hsb[:])

            # ---- exchange hs2 shards ----
            nc.gpsimd.collective_compute(
                "AllGather", AL.bypass,
                replica_groups=[list(range(NCORES))],
                ins=[hs2_shard[:]],
                outs=[hs2_full[:]],
            )

            # ---- layer 2 per dest block ----
            for b in range(BPC):
                oh2 = opool.tile([P, NT, P], DT, tag="oh")
                nc.sync.dma_start(
                    out=oh2[:], in_=oh_d[:, b * NT * P:(b + 1) * NT * P])
                msgs = []
                for hh in range(2):
                    m = mpool.tile([P, TH, F2], DT, tag=f"msg{hh}")
                    src = hs2_full[0:HALF, :] if hh == 0 else hs2_full[HALF:NPAD, :]
                    nc.gpsimd.dma_gather(
                        m[:], src,
                        idx_sb[:, (b * 2 + hh) * IW:(b * 2 + hh + 1) * IW],
                        CAP, CAP, F2, single_packet=False)
                    msgs.append(m)
                p2 = psm.tile([P, P], F32, tag="sm")
                for t in range(NT):
                    hh, tt = (0, t) if t < TH else (1, t - TH)
                    nc.tensor.matmul(p2[:], lhsT=msgs[hh][:, tt, :],
                                     rhs=oh2[:, t, :],
                                     start=(t == 0), stop=(t == NT - 1))
                o2 = spool.tile([P, P], F32, tag="o2")
                nc.vector.tensor_scalar(o2[:], p2[:], b2_sb[:, 0:1], None,
                                        AL.add)
                nc.sync.dma_start(out=out_d[:, b * P:(b + 1) * P], in_=o2[:])

    nc.compile()
    return nc


def _make_inputs(x, W1, b1, W2, b2, pp):
    TH = pp["TH"]
    CAP = TH * P
    NT = 2 * TH
    IW = CAP // 16

    xp = np.zeros((NPAD, FIN), np.float32)
    xp[:N] = x
    xbf = xp.astype(_BF16)

    # weights in device layouts
    w1c = np.ascontiguousarray(
        W1.astype(_BF16).reshape(2, P, 2, P).transpose(1, 0, 2, 3)
    ).reshape(P, 2 * 2 * P)           # [p, fin_c, h_c, h_lo]
    w2c = np.ascontiguousarray(
        W2.astype(_BF16).reshape(2, P, F2).transpose(1, 0, 2)
    ).reshape(P, 2 * F2)              # [p, h_c, f2]
    b1c = np.ascontiguousarray(
        b1.astype(np.float32).reshape(2, P).T)       # [p, h_c]
    b2c = np.ascontiguousarray(
        b2.astype(np.float32).reshape(P, 1))

    oh = pp["oh"]                     # [NB*2*TH, 128, 128] bf16
    src_slot = pp["src_slot"]         # [NB*2*CAP]
    idx_w = pp["idx_w"]               # [NB, 2, 128, IW]

    in_maps = []
    for c in range(NCORES):
        b0 = c * BPC
        sl = src_slot[b0 * 2 * CAP:(b0 + BPC) * 2 * CAP]
        xg = xbf[np.clip(sl, 0, None)]               # [BPC*NT*128, 256]
        xg = np.ascontiguousarray(
            xg.reshape(BPC * NT, P, FIN).transpose(1, 0, 2)
        ).reshape(P, BPC * NT * FIN)
        ohc = np.ascontiguousarray(
            oh[b0 * NT:(b0 + BPC) * NT].transpose(1, 0, 2)
        ).reshape(P, BPC * NT * P)
        idxc = np.ascontiguousarray(
            idx_w[b0:b0 + BPC].transpose(2, 0, 1, 3)
        ).reshape(P, BPC * 2 * IW)
        in_maps.append({
            "xg": xg, "oh": ohc, "idxP": idxc,
            "w1c": w1c, "w2c": w2c, "b1c": b1c, "b2c": b2c,
        })
    return in_maps


def kernel(x, edge_index, edge_weight, W1, b1, W2, b2, _trace=False):
    from concourse.bass_utils import run_bass_kernel_spmd

    x = np.asarray(x, dtype=np.float32)
    W1 = np.asarray(W1, dtype=np.float32)
    b1 = np.asarray(b1, dtype=np.float32)
    W2 = np.asarray(W2, dtype=np.float32)
    b2 = np.asarray(b2, dtype=np.float32)

    pp = _preprocess(np.asarray(edge_index), np.asarray(edge_weight))
    key = (pp["TH"],)
    if key not in _NC_CACHE:
        _NC_CACHE[key] = _build_nc(*key)
    nc = _NC_CACHE[key]

    in_maps = _make_inputs(x, W1, b1, W2, b2, pp)
    res = run_bass_kernel_spmd(nc, in_maps, list(range(NCORES)), trace=_trace)
    out = np.concatenate(
        [np.asarray(res.results[c]["out2T"]).T for c in range(NCORES)], axis=0)
    if _trace:
        kernel._last_result = res
    return np.ascontiguousarray(out[:N].astype(np.float32))


# revision 13
# speedup vs baseline: 2.0595x; 1.0161x over previous
"""GCN encoder (2-layer) Bass kernel for Trainium2, 8 NeuronCores.

Strategy (graph/data parallel; dest nodes sharded, contiguous ranges):
  - Nodes padded to NPAD=50176; core c owns dest blocks [c*49, (c+1)*49),
    49 blocks of 128 dests each.
  - Edges (incl. self-loops) bucketed by (dest block, src half); each bucket
    padded to TH tiles of 128 edge slots. All normalization (dinv[src] *
    w * dinv[dst]) is folded into host-precomputed one-hot tiles
    oh[slot, dst_col] (bf16), shared by both layers.
  - Layer 1 needs no runtime gather: x is a kernel input, so the per-edge
    source rows xg[slot] = x[src] are gathered ON HOST and streamed as
    dense tiles. Per dest block: aggT[f, d] = sum_t xg_tile^T oh_tile
    (PE, 2 fin chunks), then out1T[h, d] = W1c^T aggT (PE), bias+relu on
    ACT (b1 is per-partition in this transposed layout), then
    hs2[d, f2] = reluT^T W2c (PE). No transposes needed anywhere.
  - hs2 shards exchanged with AllGather; layer 2 fetches per-edge rows with
    dma_gather (128 rows/tile) and aggregates with the same one-hot tiles:
    out2T[f2, d] = sum msg^T oh. Output is written f2-major and transposed
    on host.

kernel(**inputs) takes FULL inputs, returns the FULL [50000,128] f32 output.
"""

import sys

sys.path.insert(0, "/opt/trn_rl_repo")

import numpy as np
import ml_dtypes

P = 128
NCORES = 8
BPC = 49                 # dest blocks per core
SHARD = BPC * P          # 6272
NPAD = NCORES * SHARD    # 50176
HALF = NPAD // 2         # 25088
NB = NPAD // P           # 392 dest blocks
N = 50000
FIN = 256
H = 256                  # layer-1 output width
F2 = 128                 # layer-2 output width

_BF16 = ml_dtypes.bfloat16


def _preprocess(edge_index, edge_weight):
    """Edge bucketing + all graph-structure-derived device arrays."""
    row = np.asarray(edge_index[0], dtype=np.int64)
    col = np.asarray(edge_index[1], dtype=np.int64)
    w = np.asarray(edge_weight, dtype=np.float32)

    loop = np.arange(N, dtype=np.int64)
    rows = np.concatenate([row, loop])
    cols = np.concatenate([col, loop])
    ws = np.concatenate([w, np.ones(N, np.float32)])
    EE = rows.shape[0]

    deg = np.bincount(cols, weights=ws.astype(np.float64), minlength=NPAD)
    deg = deg.astype(np.float32)
    dinv = np.where(deg > 0, 1.0 / np.sqrt(np.maximum(deg, 1e-30)), 0.0)
    dinv = dinv.astype(np.float32)
    wfull = (dinv[rows] * ws * dinv[cols]).astype(np.float32)

    blk = cols // P
    half = (rows >= HALF).astype(np.int64)
    key = blk * 2 + half                    # bucket id in [0, NB*2)

    # Sort by (bucket, src, col); fold duplicate (bucket,src,col) weights;
    # dedup (bucket, src) into slots so each distinct source is gathered once
    # per bucket (its oh row then has one entry per incident dest col).
    skey = (key * NPAD + rows) * P + (cols % P)
    order = np.argsort(skey, kind="stable")
    sk = skey[order]
    w_s = wfull[order]
    grp_first = np.ones(EE, bool)
    grp_first[1:] = sk[1:] != sk[:-1]
    gidx = np.cumsum(grp_first) - 1
    w_g = np.bincount(gidx, weights=w_s.astype(np.float64)).astype(np.float32)
    sk_g = sk[grp_first]                    # unique (bucket,src,col), sorted
    col_g = sk_g % P
    bs_g = sk_g // P                        # bucket*NPAD + src
    key_g = bs_g // NPAD
    src_g = bs_g % NPAD

    slot_first = np.ones(bs_g.shape[0], bool)
    slot_first[1:] = bs_g[1:] != bs_g[:-1]
    slot_id = np.cumsum(slot_first) - 1     # global slot per (bucket,src)
    bs_u = bs_g[slot_first]
    key_u = bs_u // NPAD
    src_u = bs_u % NPAD
    ucnt = np.bincount(key_u, minlength=NB * 2)   # unique srcs per bucket
    TH = int(-(-ucnt.max() // P))
    CAP = TH * P
    ucs = np.zeros(NB * 2 + 1, np.int64)
    np.cumsum(ucnt, out=ucs[1:])
    upos = np.arange(bs_u.shape[0]) - ucs[key_u]  # slot pos within bucket

    # per-slot src array, bucket-major (pad = -1)
    src_slot = np.full(NB * 2 * CAP, -1, np.int64)
    src_slot[key_u * CAP + upos] = src_u

    # one-hot tiles [NB*2*TH, 128 slot, 128 dst] bf16, weights folded
    oh = np.zeros((NB * 2 * TH, P, P), _BF16)
    up = upos[slot_id]                            # within-bucket slot of entry
    oh[key_g * TH + up // P, up % P, col_g] = w_g.astype(_BF16)

    # gather indices, int16, relative to half; trailing pads are -1 so the
    # ucode trims their descriptors (ring space is reserved from the
    # per-bucket count register). First 2 blocks per core keep positive pads
    # (full first-touch writes of the msg tiles).
    IW = CAP // 16
    src_rel = np.where(src_slot >= 0,
                       np.where(src_slot >= HALF, src_slot - HALF, src_slot),
                       -1).astype(np.int16)
    src_rel = src_rel.reshape(NB * 2, CAP)
    # static per-(block,half) gather count: max real count over the 8 cores
    # (the SPMD program bakes one count per gather instruction). Slots between
    # the core's real count and the static count are positive 0-pads (w=0);
    # beyond it, -1 (descriptors trimmed). First 2 blocks use full CAP
    # (first-touch full writes of the msg tiles).
    ucnt2 = ucnt.reshape(NCORES, BPC * 2)
    cnts = ucnt2.max(axis=0).astype(np.int32)       # [BPC*2]
    cnts[:2 * 2] = CAP
    for c in range(NCORES):
        for k2 in range(BPC * 2):
            k = c * BPC * 2 + k2
            row = src_rel[k]
            row[ucnt2[c, k2]:cnts[k2]] = 0
    idx_w = src_rel.reshape(NB, 2, IW, 16).transpose(0, 1, 3, 2)
    idx_w = np.ascontiguousarray(np.tile(idx_w, (1, 1, 8, 1)))  # [NB,2,128,IW]

    return dict(TH=TH, CAP=CAP, oh=oh, src_slot=src_slot, idx_w=idx_w,
                cnts=cnts, dinv=dinv)


def _host_golden(x, W1, b1, W2, b2, pp):
    """Numpy re-implementation of the device algorithm with the same bf16
    quantization points, for off-device validation."""
    bf = lambda a: a.astype(_BF16).astype(np.float32)
    TH, CAP = pp["TH"], pp["CAP"]
    oh = pp["oh"].astype(np.float32)         # [NB*2*TH, 128, 128]
    src = pp["src_slot"].reshape(NB, 2 * CAP)

    xp = np.zeros((NPAD, FIN), np.float32)
    xp[:N] = x
    xbf = bf(xp)
    W1b, W2b = bf(W1), bf(W2)

    hs2 = np.zeros((NPAD, F2), np.float32)
    for nb in range(NB):
        aggT = np.zeros((FIN, P), np.float32)
        for t in range(2 * TH):
            sl = src[nb, t * P:(t + 1) * P]
            xg = xbf[np.clip(sl, 0, None)]          # [128, 256]
            o = oh[nb * 2 * TH + t]                 # [128 slot, 128 dst]
            aggT += xg.T @ o
        aggT = bf(aggT)
        out1T = W1b.T @ aggT + b1[:, None]          # [256 h, 128 d]
        reluT = bf(np.maximum(out1T, 0.0))
        hs2[nb * P:(nb + 1) * P] = bf(reluT.T @ W2b)

    hs2b = bf(hs2)
    out = np.zeros((NPAD, F2), np.float32)
    for nb in range(NB):
        out2T = np.zeros((F2, P), np.float32)
        for t in range(2 * TH):
            sl = src[nb, t * P:(t + 1) * P]
            msg = hs2b[np.clip(sl, 0, None)]        # [128, 128]
            o = oh[nb * 2 * TH + t]
            out2T += msg.T @ o
        out[nb * P:(nb + 1) * P] = out2T.T + b2[None, :]
    return out[:N].astype(np.float32)


# ---------------------------------------------------------------------------
# Bass device kernel
# ---------------------------------------------------------------------------

_NC_CACHE = {}


def _build_nc(TH, cnts):
    import concourse.bass as bass  # noqa: F401
    import concourse.mybir as mybir
    import concourse.tile as tile
    from concourse import bacc
    from concourse.library_config import mlp

    DT = mybir.dt.bfloat16
    F32 = mybir.dt.float32
    I16 = mybir.dt.int16
    I32 = mybir.dt.int32
    AF = mybir.ActivationFunctionType
    AL = mybir.AluOpType

    CAP = TH * P
    IW = CAP // 16
    NT = 2 * TH              # tiles per dest block

    nc = bacc.Bacc("TRN2", target_bir_lowering=False, debug=True,
                   num_devices=NCORES)
    xg_d = nc.dram_tensor("xg", [P, BPC * NT * FIN], DT, kind="ExternalInput")
    oh_d = nc.dram_tensor("oh", [P, BPC * NT * P], DT, kind="ExternalInput")
    idx_d = nc.dram_tensor("idxP", [P, BPC * 2 * IW], I16, kind="ExternalInput")
    w1_d = nc.dram_tensor("w1c", [P, 2 * 2 * P], DT, kind="ExternalInput")
    w2_d = nc.dram_tensor("w2c", [P, 2 * F2], DT, kind="ExternalInput")
    b1_d = nc.dram_tensor("b1c", [P, 2], F32, kind="ExternalInput")
    b2_d = nc.dram_tensor("b2c", [P, 1], F32, kind="ExternalInput")
    out_d = nc.dram_tensor("out2T", [P, BPC * P], F32, kind="ExternalOutput")

    with tile.TileContext(nc) as tc:
        with (
            tc.tile_pool(name="dram", bufs=1, space="DRAM") as dpool,
            tc.tile_pool(name="const", bufs=1) as cpool,
            tc.tile_pool(name="xgs", bufs=2) as xpool,
            tc.tile_pool(name="ohs", bufs=2) as opool,
            tc.tile_pool(name="msg", bufs=2) as mpool,
            tc.tile_pool(name="sb", bufs=3) as spool,
            tc.tile_pool(name="pagg", bufs=4, space="PSUM") as pagg,
            tc.tile_pool(name="po1", bufs=2, space="PSUM") as po1,
            tc.tile_pool(name="psm", bufs=2, space="PSUM") as psm,
        ):
            hs2_shard = dpool.tile([SHARD, F2], DT)
            hs2_full = dpool.tile([NPAD, F2], DT, addr_space="Shared")

            nc.gpsimd.load_library(mlp)

            # ---- constants ----
            w1_sb = cpool.tile([P, 2, 2, P], DT)     # [fin_c, h_c]
            nc.sync.dma_start(out=w1_sb[:], in_=w1_d[:])
            w2_sb = cpool.tile([P, 2, F2], DT)       # [h_c]
            nc.sync.dma_start(out=w2_sb[:], in_=w2_d[:])
            b1_sb = cpool.tile([P, 2], F32)
            nc.sync.dma_start(out=b1_sb[:], in_=b1_d[:])
            b2_sb = cpool.tile([P, 1], F32)
            nc.sync.dma_start(out=b2_sb[:], in_=b2_d[:])
            idx_sb = cpool.tile([P, BPC * 2 * IW], I16)
            nc.sync.dma_start(out=idx_sb[:], in_=idx_d[:])

            # ---- layer 1 + layer-2 table, per dest block ----
            for b in range(BPC):
                xg = xpool.tile([P, NT, FIN], DT, tag="xg")
                nc.sync.dma_start(
                    out=xg[:], in_=xg_d[:, b * NT * FIN:(b + 1) * NT * FIN])
                oh = opool.tile([P, NT, P], DT, tag="oh")
                nc.sync.dma_start(
                    out=oh[:], in_=oh_d[:, b * NT * P:(b + 1) * NT * P])

                aggT0 = pagg.tile([P, P], F32, tag="aggT")
                aggT1 = pagg.tile([P, P], F32, tag="aggT")
                aggTp = [aggT0, aggT1]
                for t in range(NT):
                    for c in range(2):
                        nc.tensor.matmul(
                            aggTp[c][:],
                            lhsT=xg[:, t, c * P:(c + 1) * P],
                            rhs=oh[:, t, :],
                            start=(t == 0), stop=(t == NT - 1))
                aggT_sb = spool.tile([P, 2, P], DT, tag="aggT_sb")
                for c in range(2):
                    nc.vector.tensor_copy(aggT_sb[:, c, :], aggTp[c][:])

                reluT_sb = spool.tile([P, 2, P], DT, tag="reluT")
                for hc in range(2):
                    o1 = po1.tile([P, P], F32, tag="o1")
                    for c in range(2):
                        nc.tensor.matmul(
                            o1[:], lhsT=w1_sb[:, c, hc, :],
                            rhs=aggT_sb[:, c, :],
                            start=(c == 0), stop=(c == 1))
                    nc.scalar.activation(reluT_sb[:, hc, :], o1[:], AF.Relu,
                                         bias=b1_sb[:, hc:hc + 1])

                ph = psm.tile([P, F2], F32, tag="sm")
                for hc in range(2):
                    nc.tensor.matmul(ph[:], lhsT=reluT_sb[:, hc, :],
                                     rhs=w2_sb[:, hc, :],
                                     start=(hc == 0), stop=(hc == 1))
                hsb = spool.tile([P, F2], DT, tag="hsb")
                nc.vector.tensor_copy(hsb[:], ph[:])
                nc.sync.dma_start(out=hs2_shard[b * P:(b + 1) * P, :],
                                  in_=hsb[:])

            # ---- exchange hs2 shards ----
            nc.gpsimd.collective_compute(
                "AllGather", AL.bypass,
                replica_groups=[list(range(NCORES))],
                ins=[hs2_shard[:]],
                outs=[hs2_full[:]],
            )

            # ---- layer 2 per dest block ----
            for b in range(BPC):
                oh2 = opool.tile([P, NT, P], DT, tag="oh")
                nc.sync.dma_start(
                    out=oh2[:], in_=oh_d[:, b * NT * P:(b + 1) * NT * P])
                msgs = []
                for hh in range(2):
                    m = mpool.tile([P, TH, F2], DT, tag=f"msg{hh}")
                    src = hs2_full[0:HALF, :] if hh == 0 else hs2_full[HALF:NPAD, :]
                    k = b * 2 + hh
                    nc.gpsimd.dma_gather(
                        m[:], src,
                        idx_sb[:, k * IW:(k + 1) * IW],
                        CAP, int(cnts[k]), F2, single_packet=False)
                    msgs.append(m)
                p2 = psm.tile([P, P], F32, tag="sm")
                for t in range(NT):
                    hh, tt = (0, t) if t < TH else (1, t - TH)
                    nc.tensor.matmul(p2[:], lhsT=msgs[hh][:, tt, :],
                                     rhs=oh2[:, t, :],
                                     start=(t == 0), stop=(t == NT - 1))
                o2 = spool.tile([P, P], F32, tag="o2")
                nc.vector.tensor_scalar(o2[:], p2[:], b2_sb[:, 0:1], None,
                                        AL.add)
                nc.sync.dma_start(out=out_d[:, b * P:(b + 1) * P], in_=o2[:])

    nc.compile()
    return nc


def _make_inputs(x, W1, b1, W2, b2, pp):
    TH = pp["TH"]
    CAP = TH * P
    NT = 2 * TH
    IW = CAP // 16

    xp = np.zeros((NPAD, FIN), np.float32)
    xp[:N] = x
    xbf = xp.astype(_BF16)

    # weights in device layouts
    w1c = np.ascontiguousarray(
        W1.astype(_BF16).reshape(2, P, 2, P).transpose(1, 0, 2, 3)
    ).reshape(P, 2 * 2 * P)           # [p, fin_c, h_c, h_lo]
    w2c = np.ascontiguousarray(
        W2.astype(_BF16).reshape(2, P, F2).transpose(1, 0, 2)
    ).reshape(P, 2 * F2)              # [p, h_c, f2]
    b1c = np.ascontiguousarray(
        b1.astype(np.float32).reshape(2, P).T)       # [p, h_c]
    b2c = np.ascontiguousarray(
        b2.astype(np.float32).reshape(P, 1))

    oh = pp["oh"]                     # [NB*2*TH, 128, 128] bf16
    src_slot = pp["src_slot"]         # [NB*2*CAP]
    idx_w = pp["idx_w"]               # [NB, 2, 128, IW]

    in_maps = []
    for c in range(NCORES):
        b0 = c * BPC
        sl = src_slot[b0 * 2 * CAP:(b0 + BPC) * 2 * CAP]
        xg = xbf[np.clip(sl, 0, None)]               # [BPC*NT*128, 256]
        xg = np.ascontiguousarray(
            xg.reshape(BPC * NT, P, FIN).transpose(1, 0, 2)
        ).reshape(P, BPC * NT * FIN)
        ohc = np.ascontiguousarray(
            oh[b0 * NT:(b0 + BPC) * NT].transpose(1, 0, 2)
        ).reshape(P, BPC * NT * P)
        idxc = np.ascontiguousarray(
            idx_w[b0:b0 + BPC].transpose(2, 0, 1, 3)
        ).reshape(P, BPC * 2 * IW)
        in_maps.append({
            "xg": xg, "oh": ohc, "idxP": idxc,
            "w1c": w1c, "w2c": w2c, "b1c": b1c, "b2c": b2c,
        })
    return in_maps


def kernel(x, edge_index, edge_weight, W1, b1, W2, b2, _trace=False):
    from concourse.bass_utils import run_bass_kernel_spmd

    x = np.asarray(x, dtype=np.float32)
    W1 = np.asarray(W1, dtype=np.float32)
    b1 = np.asarray(b1, dtype=np.float32)
    W2 = np.asarray(W2, dtype=np.float32)
    b2 = np.asarray(b2, dtype=np.float32)

    pp = _preprocess(np.asarray(edge_index), np.asarray(edge_weight))
    key = (pp["TH"], tuple(int(v) for v in pp["cnts"]))
    if key not in _NC_CACHE:
        _NC_CACHE[key] = _build_nc(pp["TH"], pp["cnts"])
    nc = _NC_CACHE[key]

    in_maps = _make_inputs(x, W1, b1, W2, b2, pp)
    res = run_bass_kernel_spmd(nc, in_maps, list(range(NCORES)), trace=_trace)
    out = np.concatenate(
        [np.asarray(res.results[c]["out2T"]).T for c in range(NCORES)], axis=0)
    if _trace:
        kernel._last_result = res
    return np.ascontiguousarray(out[:N].astype(np.float32))
